# revision 5
# baseline (speedup 1.0000x reference)
"""Trainium2 Bass kernel for Chn8ActGrp3WgtQuantizedLinear.

Computes: out = fake_quant8_per_row(x) @ dequant(weight_qvals, weight_scales).T

  x:             (1024, 4096)  f32
  weight_qvals:  (11008, 4096) int32, 3-bit values in [-4, 3]
  weight_scales: (11008, 32)   f32, one scale per (out-channel, 128-group)
  out:           (1024, 11008) f32
  group_size:    128

Strategy (tensor parallel over 8 NeuronCores; N=11008 -> 1376/core):
  - host repack (layout/dtype only): x -> fp16; wq -> K-major fp16
    [4096, 1376] (3-bit values exact in fp16); ws -> fp16 compact
    [16*8, 2752]: per k-group-pair block b, the row concat(ws[:,2b],
    ws[:,2b+1]) replicated on just 8 partitions (0.7 MB vs 11.3 MB for a
    full 128-row broadcast).
  - device per core:
      * ws broadcast 8->128 partitions with 4 doubling SBUF->SBUF DMAs on
        the gpsimd queue; dequant W[k,n] = wq * ws_bc on DVE (fp16 2x).
      * activation fake-quant per 128-row m-tile: min/max row stats (DVE
        reduces on fp16), scale/inv; then a = round(x*inv) via the
        fp32 magic-number trick in two ACT passes
        (u = x*inv + 1.5*2^23; a = u - 1.5*2^23 -> fp16 integer).
        The clip to [qmin - z, qmax - z] is dropped: without clipping the
        zero-point cancels algebraically (a = round(x/s)), and round(x/s)
        can escape the clip range only by 1 lsb on knife-edge row extremes
        (two independent roundings), which perturbs a handful of elements
        by one quant step — far below the error budget.
      * aT[k, m] via ONE DMA-xbar transpose per m-tile
        (dma_start_transpose a[128m, 4096k] -> aT[128k', 32g, 128m]), so
        the PE array runs matmuls only.
      * matmul: psum[m=128, n=1376] += aT[:,g,:].T @ W[k,n] over 32
        k-groups (512-col psum-bank chunks); m0/m1 group-interleaved so
        the W DMA/dequant ramp is consumed at 2 m-tiles per group.
      * evict with per-row scale: out = psum * scale[m] (ACT), DMA out.
  - host concatenates the 8 (1024, 1376) shards.
"""

import sys
import types

import numpy as np

M, K, N, GS = 1024, 4096, 11008, 128
NCORES = 8
NC_SHARD = N // NCORES  # 1376
NGRP = K // GS  # 32
NBLK = NGRP // 2  # 16 k-group-pair blocks for the ws stream
MTILES = M // 128  # 8
MAGIC = 12582912.0  # 1.5 * 2**23: f32 add+sub rounds to integer (RNE)
WS_REP = 8  # ws host replication rows per block

_CACHE = {}
LAST_RESULTS = None


def _install_axon_ntff_hook():
    """Register the NTFF profile hook if the container's antenv lacks it.

    Only needed for trace=True (BASS_TRACE=1); degrades silently."""
    try:
        if "antenv.axon_hooks" in sys.modules:
            return
        import antenv

        mod = types.ModuleType("antenv.axon_hooks")
        _state = {"hook": None}
        mod.set_axon_ntff_profile_hook = lambda h: _state.__setitem__("hook", h)
        mod.get_axon_ntff_profile_hook = lambda: _state["hook"]
        sys.modules["antenv.axon_hooks"] = mod
        antenv.axon_hooks = mod

        from trn_agent_boot.trn_boot import _ntff_profile_via_ctypes

        mod.set_axon_ntff_profile_hook(
            _ntff_profile_via_ctypes("/opt/axon/libaxon_pjrt.so")
        )
    except Exception:
        pass


def _build():
    if "nc" in _CACHE:
        return _CACHE["nc"]

    import contextlib

    import concourse.bass as bass
    import concourse.tile as tile
    from concourse import bacc, mybir

    dt = mybir.dt
    F32, F16 = dt.float32, dt.float16
    ALU = mybir.AluOpType
    ACTF = mybir.ActivationFunctionType
    AX = mybir.AxisListType

    nc = bacc.Bacc("TRN2", target_bir_lowering=False, debug=False,
                   num_devices=NCORES)

    x_d = nc.dram_tensor("x", [M, K], F16, kind="ExternalInput").ap()
    wq_d = nc.dram_tensor("wq", [K, NC_SHARD], F16, kind="ExternalInput").ap()
    ws_d = nc.dram_tensor("ws", [NBLK * WS_REP, 2 * NC_SHARD], F16,
                          kind="ExternalInput").ap()
    out_d = nc.dram_tensor("out", [M, NC_SHARD], F32, kind="ExternalOutput").ap()

    CHUNKS = [(c, min(512, NC_SHARD - c)) for c in range(0, NC_SHARD, 512)]

    with tile.TileContext(nc) as tc:
        ctx = contextlib.ExitStack()
        with ctx:
            consts = ctx.enter_context(tc.tile_pool(name="consts", bufs=1))
            wpool = ctx.enter_context(tc.tile_pool(name="w", bufs=1))
            wqld = ctx.enter_context(tc.tile_pool(name="wqld", bufs=3))
            wsb = ctx.enter_context(tc.tile_pool(name="ws", bufs=2))
            xp = ctx.enter_context(tc.tile_pool(name="x", bufs=3))
            up = ctx.enter_context(tc.tile_pool(name="u", bufs=1))
            ap_ = ctx.enter_context(tc.tile_pool(name="a", bufs=2))
            atp = ctx.enter_context(tc.tile_pool(name="at", bufs=3))
            outp = ctx.enter_context(tc.tile_pool(name="o", bufs=2))
            vecs = ctx.enter_context(tc.tile_pool(name="v", bufs=8))
            ps_out = ctx.enter_context(
                tc.tile_pool(name="pso", bufs=2, space="PSUM"))

            magic_vec = consts.tile([128, 1], F32)
            nc.vector.memset(magic_vec[:], MAGIC)
            neg_magic_vec = consts.tile([128, 1], F32)
            nc.vector.memset(neg_magic_vec[:], -MAGIC)

            # W holds all dequantized weights, k-major: [k%128, g, n]
            W = wpool.tile([128, NGRP * NC_SHARD], F16)

            x_of = {}
            scale_of = {}
            inv_of = {}
            at_of = {}

            def load_x(m, chunked=False):
                x_t = xp.tile([128, K], F16, tag="xt")
                if chunked:  # startup: stats can begin before full row lands
                    for j in range(4):
                        sl = slice(j * 1024, (j + 1) * 1024)
                        nc.scalar.dma_start(x_t[:, sl],
                                            x_d[m * 128:(m + 1) * 128, sl])
                else:
                    nc.scalar.dma_start(x_t[:], x_d[m * 128:(m + 1) * 128, :])
                x_of[m] = x_t

            def stats(m, chunked=False):
                """DVE row min/max -> scale, inv."""
                x_t = x_of[m]
                mx = vecs.tile([128, 1], F32, tag="mx")
                mn = vecs.tile([128, 1], F32, tag="mn")
                if chunked:
                    mxp = vecs.tile([128, 4], F32, tag="mxp")
                    mnp = vecs.tile([128, 4], F32, tag="mnp")
                    for j in range(4):
                        sl = slice(j * 1024, (j + 1) * 1024)
                        nc.vector.tensor_reduce(mxp[:, j:j + 1], x_t[:, sl],
                                                axis=AX.X, op=ALU.max)
                        nc.vector.tensor_reduce(mnp[:, j:j + 1], x_t[:, sl],
                                                axis=AX.X, op=ALU.min)
                    nc.vector.tensor_reduce(mx[:], mxp[:], axis=AX.X, op=ALU.max)
                    nc.vector.tensor_reduce(mn[:], mnp[:], axis=AX.X, op=ALU.min)
                else:
                    nc.vector.tensor_reduce(mx[:], x_t[:], axis=AX.X, op=ALU.max)
                    nc.vector.tensor_reduce(mn[:], x_t[:], axis=AX.X, op=ALU.min)
                xc = vecs.tile([128, 1], F32, tag="xc")
                nc.vector.tensor_scalar(xc[:], mx[:], 0.0, None, ALU.max)
                nn_ = vecs.tile([128, 1], F32, tag="nn")
                nc.vector.tensor_scalar(nn_[:], mn[:], 0.0, None, ALU.min)
                df = vecs.tile([128, 1], F32, tag="df")
                nc.vector.tensor_tensor(df[:], xc[:], nn_[:], ALU.subtract)
                sc = vecs.tile([128, 1], F32, tag="sc")
                nc.vector.tensor_scalar(sc[:], df[:], 1.0 / 255.0, 1e-9,
                                        ALU.mult, ALU.max)
                inv = vecs.tile([128, 1], F32, tag="inv")
                nc.vector.reciprocal(inv[:], sc[:])
                scale_of[m] = sc
                inv_of[m] = inv

            def quant_acts(m):
                """ACT: u = x*inv + MAGIC (rounds, RNE); a = u - MAGIC -> f16."""
                u = up.tile([128, K], F32, tag="u")
                nc.scalar.activation(u[:], x_of[m][:], ACTF.Identity,
                                     bias=magic_vec[:], scale=inv_of[m][:])
                a_t = ap_.tile([128, K], F16, tag="a")
                nc.scalar.activation(a_t[:], u[:], ACTF.Identity,
                                     bias=neg_magic_vec[:], scale=1.0)
                return a_t

            def trT(m, a_t):
                """One DMA-xbar transpose: a[128m, (g k')] -> aT[128k', g, 128m]."""
                aT = atp.tile([128, NGRP, 128], F16, tag="aT")
                nc.scalar.dma_start_transpose(aT[:], a_t[:])
                at_of[m] = aT

            def quant_phase(m, chunked=False):
                stats(m, chunked)
                trT(m, quant_acts(m))

            def ws_block(b):
                """Compact ws rows in; broadcast 8 -> 128 partitions (gpsimd DMAs)."""
                ws_bc = wsb.tile([128, 2 * NC_SHARD], F16, tag="wsb")
                nc.sync.dma_start(ws_bc[0:WS_REP, :],
                                  ws_d[b * WS_REP:(b + 1) * WS_REP, :])
                p = WS_REP
                while p < 128:
                    nc.gpsimd.dma_start(ws_bc[p:2 * p, :], ws_bc[0:p, :])
                    p *= 2
                return ws_bc

            def wq_deq(g, ws_bc, j):
                wq_t = wqld.tile([128, NC_SHARD], F16, tag="wq")
                nc.sync.dma_start(wq_t[:], wq_d[g * 128:(g + 1) * 128, :])
                nc.vector.tensor_tensor(
                    W[:, g * NC_SHARD:(g + 1) * NC_SHARD], wq_t[:],
                    ws_bc[:, j * NC_SHARD:(j + 1) * NC_SHARD], ALU.mult)

            def mm_group(psum, aT, g):
                for (c0, cw) in CHUNKS:
                    nc.tensor.matmul(psum[:, c0:c0 + cw],
                                     lhsT=aT[:, g, :],
                                     rhs=W[:, g * NC_SHARD + c0:
                                           g * NC_SHARD + c0 + cw],
                                     start=(g == 0), stop=(g == NGRP - 1))

            def evict(m, psum):
                o_t = outp.tile([128, NC_SHARD], F32, tag="o")
                nc.scalar.activation(o_t[:], psum[:], ACTF.Identity,
                                     bias=0.0, scale=scale_of[m][:])
                nc.sync.dma_start(out_d[m * 128:(m + 1) * 128, :], o_t[:])

            # ---- emission ----
            load_x(0, chunked=True)
            load_x(1)
            load_x(2)
            quant_phase(0, chunked=True)
            quant_phase(1)

            # W stream: ws block + 2 wq groups + dequant per iteration.
            # quant for m2/m3 interleaved into the DVE/ACT streams mid-ramp.
            for b in range(NBLK):
                ws_bc = ws_block(b)
                for j in range(2):
                    wq_deq(2 * b + j, ws_bc, j)
                if b == 3:
                    load_x(3)
                    quant_phase(2)
                if b == 7:
                    load_x(4)
                    quant_phase(3)

            # fused m0+m1 matmul ramp: both consume each W group as it lands
            ps0 = ps_out.tile([128, NC_SHARD], F32, tag="psum")
            ps1 = ps_out.tile([128, NC_SHARD], F32, tag="psum")
            for g in range(NGRP):
                mm_group(ps0, at_of[0], g)
                mm_group(ps1, at_of[1], g)
            evict(0, ps0)
            evict(1, ps1)

            for m in range(2, MTILES):
                psum = ps_out.tile([128, NC_SHARD], F32, tag="psum")
                for g in range(NGRP):
                    mm_group(psum, at_of[m], g)
                evict(m, psum)
                if m + 3 < MTILES:
                    load_x(m + 3)
                if m + 2 < MTILES:
                    quant_phase(m + 2)

    nc.compile()
    _CACHE["nc"] = nc
    return nc


def kernel(x, weight_qvals, weight_scales, group_size):
    global LAST_RESULTS
    _install_axon_ntff_hook()
    from concourse.bass_utils import run_bass_kernel_spmd

    x = np.asarray(x, dtype=np.float32)
    wq = np.asarray(weight_qvals)
    ws = np.asarray(weight_scales, dtype=np.float32)
    assert int(group_size) == GS
    assert x.shape == (M, K) and wq.shape == (N, K) and ws.shape == (N, NGRP)

    nc = _build()

    x16 = x.astype(np.float16)
    in_maps = []
    for c in range(NCORES):
        sl = slice(c * NC_SHARD, (c + 1) * NC_SHARD)
        wq_c = np.ascontiguousarray(wq[sl].T).astype(np.float16)
        # ws rows per block b: concat(ws[:,2b], ws[:,2b+1]), replicated x8
        ws_t = ws[sl].T.astype(np.float16)  # [32, 1376]
        ws_rows = ws_t.reshape(NBLK, 2 * NC_SHARD)
        ws_c = np.ascontiguousarray(
            np.broadcast_to(ws_rows[:, None, :], (NBLK, WS_REP, 2 * NC_SHARD))
        ).reshape(NBLK * WS_REP, 2 * NC_SHARD)
        in_maps.append({"x": x16, "wq": wq_c, "ws": ws_c})

    res = run_bass_kernel_spmd(nc, in_maps, core_ids=list(range(NCORES)))
    LAST_RESULTS = res
    out = np.concatenate([r["out"] for r in res.results], axis=1)
    return out


if __name__ == "__main__":
    rng = np.random.default_rng(0)
    xv = rng.standard_normal((M, K)).astype(np.float32)
    wqv = rng.integers(-4, 4, (N, K)).astype(np.int32)
    wsv = (rng.random((N, NGRP)).astype(np.float32) * 0.02 + 1e-4)
    o = kernel(xv, wqv, wsv, GS)
    print("out shape:", o.shape, "finite:", np.isfinite(o).all())
